# revision 5
# baseline (speedup 1.0000x reference)
"""Trainium2 Bass/Tile kernel for nn_Capsule_6004364280312.

Computes CapsNet dynamic routing:
    u_hat = einsum('bnd,dm->bnm', u_vecs, W[0]) reshaped to [B, NC, N, DC]
    3 rounds of routing (softmax over N / weighted sum / squash / agreement)
    returns v [B, NC, DC]

Strategy (per core, batch-parallel over 8 cores, 4 batches each):
  * never materialize u_hat (268 MB). Algebra:
        s[i]  = (e[i] @ u) @ W_i          (e = exp(b), unnormalized softmax)
        b[i] += u @ (W_i @ (s[i] * rsqrt(||s[i]||^2 + eps)))
    The softmax normalizer cancels: v = normalize(s) is invariant to row
    scaling of e, so softmax is just exp().
  * partition layout p = bl*32 + i  (bl = local batch 0..3, i = capsule 0..31)
    so per-round tensors are full-width [128, *].
  * block-diagonal extraction of s from the full [128, 2048] product via a
    small DRAM bounce with a strided (diagonal) access pattern.
"""

import numpy as np
from contextlib import ExitStack

import concourse.bass as bass
import concourse.mybir as mybir
import concourse.tile as tile
from concourse import bacc, bass_utils
from concourse.masks import make_identity

F32 = mybir.dt.float32
AF = mybir.ActivationFunctionType
ALU = mybir.AluOpType

B, N, D = 32, 1024, 256
NC, DC = 32, 64
M = NC * DC  # 2048
N_CORES = 8
BL = B // N_CORES  # local batches per core
P = 128
EPS = 1e-7
ROUTINGS = 3


def _build_kernel():
    nc = bacc.Bacc("TRN2", target_bir_lowering=False, debug=False,
                   num_devices=N_CORES)
    u_d = nc.dram_tensor("u", (BL * N, D), F32, kind="ExternalInput").ap()
    w_d = nc.dram_tensor("w", (D, M), F32, kind="ExternalInput").ap()
    v_d = nc.dram_tensor("v", (P, DC), F32, kind="ExternalOutput").ap()
    sf_d = nc.dram_tensor("sf_scratch", (P, M), F32, kind="Internal").ap()

    with tile.TileContext(nc) as tc:
        with ExitStack() as ctx:
            _body(ctx, tc, v_d, u_d, w_d, sf_d)
    nc.compile()
    return nc


def _body(ctx, tc, v_d, u_d, w_d, sf_d):
    nc = tc.nc
    const = ctx.enter_context(tc.tile_pool(name="const", bufs=1))
    work = ctx.enter_context(tc.tile_pool(name="work", bufs=2))
    ptr = ctx.enter_context(tc.tile_pool(name="ptr", bufs=2, space="PSUM"))
    pmm = ctx.enter_context(tc.tile_pool(name="pmm", bufs=2, space="PSUM"))
    pbig = ctx.enter_context(tc.tile_pool(name="pbig", bufs=1, space="PSUM"))

    # ---------------- persistent SBUF state ----------------
    ident = const.tile([P, P], F32)
    make_identity(nc, ident)
    ones32 = const.tile([P, 32], F32)
    nc.vector.memset(ones32, 1.0)
    eps_sb = const.tile([P, 1], F32)
    nc.vector.memset(eps_sb, EPS)

    u_sb = const.tile([P, BL * 8 * D], F32)    # u[bl][jk]: [128(j), 256(d)]
    uT_sb = const.tile([P, BL * 2 * N], F32)   # uT[bl][dk]: [128(d), 1024(j)]
    w_sb = const.tile([P, 2 * M], F32)         # w[dk]: [128(d), 2048(m)]
    wT_sb = const.tile([P, 16 * D], F32)       # wT[mk]: [128(m), 256(d)]
    e_sb = const.tile([P, N], F32)             # exp(b)
    b_sb = const.tile([P, N], F32)             # routing logits
    eT_sb = const.tile([P, 8 * P], F32)        # eT[jk]: [128(j), 128(p)]
    vemb = const.tile([P, 16 * P], F32)        # block-diag s embedding
    nc.vector.memset(vemb, 0.0)

    def tr(dst_ap, src_ap):
        """128x128 transpose src -> psum -> dst (scalar-engine copy out)."""
        pt = ptr.tile([P, P], F32, tag="tr")
        nc.tensor.transpose(out=pt[:], in_=src_ap, identity=ident[:])
        nc.scalar.copy(out=dst_ap, in_=pt[:])

    # ---------------- load W + build W^T ----------------
    for dk in range(2):
        nc.sync.dma_start(out=w_sb[:, dk * M:(dk + 1) * M],
                          in_=w_d[dk * 128:(dk + 1) * 128, :])
    for mk in range(16):
        for dk in range(2):
            tr(wT_sb[:, mk * D + dk * 128: mk * D + (dk + 1) * 128],
               w_sb[:, dk * M + mk * 128: dk * M + (mk + 1) * 128])

    # ---------------- load u + build u^T ----------------
    for bl in range(BL):
        for jk in range(8):
            off = (bl * 8 + jk) * D
            nc.sync.dma_start(
                out=u_sb[:, off:off + D],
                in_=u_d[bl * N + jk * 128: bl * N + (jk + 1) * 128, :])
            for dk in range(2):
                tr(uT_sb[:, (bl * 2 + dk) * N + jk * 128:
                         (bl * 2 + dk) * N + (jk + 1) * 128],
                   u_sb[:, off + dk * 128: off + (dk + 1) * 128])

    # ---------------- routing rounds ----------------
    for r in range(ROUTINGS):
        # cu[p, d] = sum_j e[p, j] * u[bl(p)][j, d]
        cu_ps = pmm.tile([P, D], F32, tag="mm")
        for bl in range(BL):
            for k in range(8):
                lhs = (ones32[:, 0:32] if r == 0 else
                       eT_sb[:, k * P + bl * 32: k * P + bl * 32 + 32])
                nc.tensor.matmul(
                    out=cu_ps[bl * 32:(bl + 1) * 32, :],
                    lhsT=lhs,
                    rhs=u_sb[:, (bl * 8 + k) * D:(bl * 8 + k + 1) * D],
                    start=(k == 0), stop=(k == 7),
                    tile_position=(0, bl * 32))
        cu_sb = work.tile([P, D], F32, tag="cu")
        nc.scalar.copy(out=cu_sb[:], in_=cu_ps[:])
        cuT_sb = work.tile([P, D], F32, tag="cuT")
        for dk in range(2):
            tr(cuT_sb[:, dk * 128:(dk + 1) * 128],
               cu_sb[:, dk * 128:(dk + 1) * 128])

        # s_full[p, m] = sum_d cu[p, d] * W[d, m]
        sf_ps = pbig.tile([P, M], F32, tag="big")
        for n in range(4):
            for dk in range(2):
                nc.tensor.matmul(
                    out=sf_ps[:, n * 512:(n + 1) * 512],
                    lhsT=cuT_sb[:, dk * 128:(dk + 1) * 128],
                    rhs=w_sb[:, dk * M + n * 512: dk * M + (n + 1) * 512],
                    start=(dk == 0), stop=(dk == 1))
        sf_sb = work.tile([P, M], F32, tag="sf")
        nc.vector.tensor_copy(out=sf_sb[:, 0:1024], in_=sf_ps[:, 0:1024])
        nc.scalar.copy(out=sf_sb[:, 1024:2048], in_=sf_ps[:, 1024:2048])

        # bounce through DRAM to extract diagonal blocks:
        # s[p, d'] = s_full[p, i(p)*64 + d'],  i(p) = p % 32
        nc.sync.dma_start(out=sf_d[:], in_=sf_sb[:])
        s_sb = work.tile([P, DC], F32, tag="s")
        for bl in range(BL):
            src = bass.AP(tensor=sf_d.tensor, offset=bl * 32 * M,
                          ap=[[M + DC, 32], [1, DC]])
            nc.sync.dma_start(out=s_sb[bl * 32:(bl + 1) * 32, :], in_=src)

        # squash scale: rv = 1/sqrt(sum(s^2) + eps)
        sq_sb = work.tile([P, DC], F32, tag="sq")
        nc.vector.tensor_mul(sq_sb[:], s_sb[:], s_sb[:])
        ssq = work.tile([P, 1], F32, tag="ssq")
        nc.vector.reduce_sum(out=ssq[:], in_=sq_sb[:], axis=mybir.AxisListType.X)
        sr = work.tile([P, 1], F32, tag="sr")
        nc.scalar.activation(out=sr[:], in_=ssq[:], func=AF.Sqrt,
                             bias=eps_sb[:])
        rv = work.tile([P, 1], F32, tag="rv")
        nc.vector.reciprocal(out=rv[:], in_=sr[:])

        if r == ROUTINGS - 1:
            v_sb = work.tile([P, DC], F32, tag="v")
            nc.vector.tensor_scalar(out=v_sb[:], in0=s_sb[:],
                                    scalar1=rv[:, 0:1], scalar2=None,
                                    op0=ALU.mult)
            nc.sync.dma_start(out=v_d[:], in_=v_sb[:])
            continue

        # s2 = [s, s] duplicated along free dim; s2T[t*64+d', p] = s[p, d']
        s2_sb = work.tile([P, 2 * DC], F32, tag="s2")
        nc.scalar.copy(out=s2_sb[:, 0:DC], in_=s_sb[:])
        nc.scalar.copy(out=s2_sb[:, DC:2 * DC], in_=s_sb[:])
        s2T_sb = work.tile([P, P], F32, tag="s2T")
        tr(s2T_sb[:], s2_sb[:])

        # scatter s into the block-diagonal embedding vemb[m_tile k]:
        # vemb_k[t*64+d', p] = s[p, d'] for p with capsule i(p) == 2k+t
        for k in range(16):
            for t in range(2):
                i_cap = 2 * k + t
                rows = slice(t * 64, (t + 1) * 64)
                src = s2T_sb[rows, :].rearrange("p (b c) -> p b c", b=4)[:, :, i_cap]
                dst = vemb[rows, k * P:(k + 1) * P].rearrange(
                    "p (b c) -> p b c", b=4)[:, :, i_cap]
                nc.vector.tensor_copy(out=dst, in_=src)

        # w_v[p, d] = sum_{d'} s[p, d'] * W[d, i(p)*64+d']  (then scaled by rv)
        wv_ps = pmm.tile([P, D], F32, tag="mm")
        for k in range(16):
            nc.tensor.matmul(out=wv_ps[:],
                             lhsT=vemb[:, k * P:(k + 1) * P],
                             rhs=wT_sb[:, k * D:(k + 1) * D],
                             start=(k == 0), stop=(k == 15))
        wv_sb = work.tile([P, D], F32, tag="wv")
        nc.vector.tensor_scalar(out=wv_sb[:], in0=wv_ps[:],
                                scalar1=rv[:, 0:1], scalar2=None, op0=ALU.mult)
        wvT_sb = work.tile([P, D], F32, tag="wvT")
        for dk in range(2):
            tr(wvT_sb[:, dk * 128:(dk + 1) * 128],
               wv_sb[:, dk * 128:(dk + 1) * 128])

        # bu[p, j] = sum_d wv[p, d] * u[bl(p)][j, d]
        bu_ps = pbig.tile([P, N], F32, tag="big")
        for bl in range(BL):
            for n in range(2):
                for dk in range(2):
                    nc.tensor.matmul(
                        out=bu_ps[bl * 32:(bl + 1) * 32, n * 512:(n + 1) * 512],
                        lhsT=wvT_sb[:, dk * 128 + bl * 32: dk * 128 + bl * 32 + 32],
                        rhs=uT_sb[:, (bl * 2 + dk) * N + n * 512:
                                  (bl * 2 + dk) * N + (n + 1) * 512],
                        start=(dk == 0), stop=(dk == 1),
                        tile_position=(0, bl * 32))

        # b += bu ; e = exp(b)
        if r == 0:
            nc.vector.tensor_copy(out=b_sb[:], in_=bu_ps[:])
        else:
            nc.vector.tensor_add(out=b_sb[:], in0=b_sb[:], in1=bu_ps[:])
        nc.scalar.activation(out=e_sb[:], in_=b_sb[:], func=AF.Exp)
        for jk in range(8):
            tr(eT_sb[:, jk * P:(jk + 1) * P],
               e_sb[:, jk * 128:(jk + 1) * 128])


_NC_CACHE = None


def _get_nc():
    global _NC_CACHE
    if _NC_CACHE is None:
        _NC_CACHE = _build_kernel()
    return _NC_CACHE


def kernel(u_vecs: np.ndarray, W: np.ndarray) -> np.ndarray:
    u_vecs = np.ascontiguousarray(np.asarray(u_vecs, dtype=np.float32))
    W0 = np.ascontiguousarray(np.asarray(W, dtype=np.float32).reshape(D, M))
    nc = _get_nc()
    in_maps = [
        {"u": u_vecs[c * BL:(c + 1) * BL].reshape(BL * N, D), "w": W0}
        for c in range(N_CORES)
    ]
    res = bass_utils.run_bass_kernel_spmd(nc, in_maps,
                                          core_ids=list(range(N_CORES)))
    out = np.empty((B, NC, DC), dtype=np.float32)
    for c in range(N_CORES):
        out[c * BL:(c + 1) * BL] = res.results[c]["v"].reshape(BL, NC, DC)
    return out


# revision 7
# speedup vs baseline: 1.4741x; 1.4741x over previous
"""Trainium2 Bass/Tile kernel for nn_Capsule_6004364280312.

Computes CapsNet dynamic routing:
    u_hat = einsum('bnd,dm->bnm', u_vecs, W[0]) reshaped to [B, NC, N, DC]
    3 rounds of routing (softmax over N / weighted sum / squash / agreement)
    returns v [B, NC, DC]

Strategy (per core, batch-parallel over 8 cores, 4 batches each):
  * never materialize u_hat (268 MB). Algebra:
        s[i]  = (e[i] @ u) @ W_i          (e = exp(b), unnormalized softmax)
        b[i] += u @ (W_i @ (s[i] * rsqrt(||s[i]||^2 + eps)))
    The softmax normalizer cancels: v = normalize(s) is invariant to row
    scaling of e, so softmax is just exp().
  * partition layout p = bl*32 + i  (bl = local batch 0..3, i = capsule 0..31)
    so per-round tensors are full-width [128, *].
  * all matmul operands in float32r (TF32-like, 4x faster PE than fp32;
    measured rel err ~1.5e-4 per matmul, final ~3e-4 << resid_var gate).
    f32r matmuls require dst partition base 0, so the per-batch (cu) and
    per-batch (bu) contractions use block-masked weights over the
    concatenated contraction axis instead of col-tiled partial outputs.
  * block-diagonal extraction of s from the full [128, 2048] product via a
    small DRAM bounce with a strided (diagonal) access pattern.
"""

import numpy as np
from contextlib import ExitStack

import concourse.bass as bass
import concourse.mybir as mybir
import concourse.tile as tile
from concourse import bacc, bass_utils
from concourse.masks import make_identity

F32 = mybir.dt.float32
F32R = mybir.dt.float32r
AF = mybir.ActivationFunctionType
ALU = mybir.AluOpType

B, N, D = 32, 1024, 256
NC, DC = 32, 64
M = NC * DC  # 2048
N_CORES = 8
BL = B // N_CORES  # local batches per core
P = 128
EPS = 1e-7
ROUTINGS = 3


def _build_kernel():
    nc = bacc.Bacc("TRN2", target_bir_lowering=False, debug=False,
                   num_devices=N_CORES)
    u_d = nc.dram_tensor("u", (BL * N, D), F32, kind="ExternalInput").ap()
    w_d = nc.dram_tensor("w", (D, M), F32, kind="ExternalInput").ap()
    v_d = nc.dram_tensor("v", (P, DC), F32, kind="ExternalOutput").ap()
    sf_d = nc.dram_tensor("sf_scratch", (P, M), F32, kind="Internal").ap()

    with tile.TileContext(nc) as tc:
        with ExitStack() as ctx:
            _body(ctx, tc, v_d, u_d, w_d, sf_d)
    nc.compile()
    return nc


def _body(ctx, tc, v_d, u_d, w_d, sf_d):
    nc = tc.nc
    const = ctx.enter_context(tc.tile_pool(name="const", bufs=1))
    work = ctx.enter_context(tc.tile_pool(name="work", bufs=2))
    ptr_r = ctx.enter_context(tc.tile_pool(name="ptr_r", bufs=2, space="PSUM"))
    ptr_f = ctx.enter_context(tc.tile_pool(name="ptr_f", bufs=2, space="PSUM"))
    pmm = ctx.enter_context(tc.tile_pool(name="pmm", bufs=2, space="PSUM"))
    pbig = ctx.enter_context(tc.tile_pool(name="pbig", bufs=1, space="PSUM"))

    # ---------------- persistent SBUF state ----------------
    ident = const.tile([P, P], F32)
    make_identity(nc, ident)
    ident_r = const.tile([P, P], F32R)
    nc.vector.tensor_copy(out=ident_r[:], in_=ident[:])
    eps_sb = const.tile([P, 1], F32)
    nc.vector.memset(eps_sb, EPS)

    # block-masked all-ones weights for round 0 (uniform softmax):
    # onesm[bl] = [128, 128] with cols [32bl, 32bl+32) = 1, else 0
    onesm = const.tile([P, BL * P], F32R)
    nc.vector.memset(onesm[:].bitcast(F32), 0.0)
    for bl in range(BL):
        nc.vector.memset(
            onesm[:, bl * P + bl * 32: bl * P + bl * 32 + 32].bitcast(F32), 1.0)

    u_sb = const.tile([P, BL * 8 * D], F32R)   # u[bl][jk]: [128(j), 256(d)]
    uT_sb = const.tile([P, BL * 2 * N], F32R)  # uT[bl][dk]: [128(d), 1024(j)]
    w_sb = const.tile([P, 2 * M], F32R)        # w[dk]: [128(d), 2048(m)]
    wT_sb = const.tile([P, 16 * D], F32R)      # wT[mk]: [128(m), 256(d)]
    b_sb = const.tile([P, N], F32)             # routing logits
    # block-masked exp(b)^T: eTm[(bl,jk)][j_local, p] = e[p, jk*128+j_local]
    # for p in bl's block, else 0
    eTm = const.tile([P, BL * 8 * P], F32R)
    nc.vector.memset(eTm[:].bitcast(F32), 0.0)
    # block-masked wv^T: wvm[(bl,dk)][d_local, p] masked to bl's block
    wvm = const.tile([P, BL * 2 * P], F32R)
    nc.vector.memset(wvm[:].bitcast(F32), 0.0)
    vemb = const.tile([P, 16 * P], F32R)       # block-diag s embedding
    nc.vector.memset(vemb[:].bitcast(F32), 0.0)

    copy_engines = [nc.scalar.copy, nc.vector.tensor_copy]

    def tr_r(dst_ap, src_ap, eng):
        """f32r 128x128 transpose src -> psum -> dst."""
        pt = ptr_r.tile([P, P], F32R, tag="trr")
        nc.tensor.transpose(out=pt[:], in_=src_ap, identity=ident_r[:])
        copy_engines[eng % 2](out=dst_ap, in_=pt[:])

    # ---------------- load W (cast to f32r) + build W^T ----------------
    for dk in range(2):
        nc.gpsimd.dma_start(out=w_sb[:, dk * M:(dk + 1) * M],
                            in_=w_d[dk * 128:(dk + 1) * 128, :])
    for mk in range(16):
        for dk in range(2):
            tr_r(wT_sb[:, mk * D + dk * 128: mk * D + (dk + 1) * 128],
                 w_sb[:, dk * M + mk * 128: dk * M + (mk + 1) * 128],
                 eng=mk * 2 + dk)

    # ---------------- load u (cast to f32r) + build u^T ----------------
    for bl in range(BL):
        for jk in range(8):
            off = (bl * 8 + jk) * D
            nc.gpsimd.dma_start(
                out=u_sb[:, off:off + D],
                in_=u_d[bl * N + jk * 128: bl * N + (jk + 1) * 128, :])
            for dk in range(2):
                tr_r(uT_sb[:, (bl * 2 + dk) * N + jk * 128:
                           (bl * 2 + dk) * N + (jk + 1) * 128],
                     u_sb[:, off + dk * 128: off + (dk + 1) * 128],
                     eng=jk * 2 + dk)

    # ---------------- routing rounds ----------------
    for r in range(ROUTINGS):
        # cu[p, d] = sum_j e[p, j] * u[bl(p)][j, d] as one accumulation over
        # the concatenated (bl, jk) axis with block-masked weights
        cu_ps = pmm.tile([P, D], F32, tag="mm")
        first, last = (0, 0), (BL - 1, 7)
        for bl in range(BL):
            for jk in range(8):
                lhs = (onesm[:, bl * P:(bl + 1) * P] if r == 0 else
                       eTm[:, (bl * 8 + jk) * P:(bl * 8 + jk + 1) * P])
                nc.tensor.matmul(
                    out=cu_ps[:],
                    lhsT=lhs,
                    rhs=u_sb[:, (bl * 8 + jk) * D:(bl * 8 + jk + 1) * D],
                    start=((bl, jk) == first), stop=((bl, jk) == last))
        cu_sb = work.tile([P, D], F32R, tag="cu")
        nc.scalar.copy(out=cu_sb[:], in_=cu_ps[:])
        cuT_sb = work.tile([P, D], F32R, tag="cuT")
        for dk in range(2):
            tr_r(cuT_sb[:, dk * 128:(dk + 1) * 128],
                 cu_sb[:, dk * 128:(dk + 1) * 128], eng=dk)

        # s_full[p, m] = sum_d cu[p, d] * W[d, m]; two psum halves,
        # bounce each through DRAM to extract diagonal blocks:
        # s[p, d'] = s_full[p, i(p)*64 + d'],  i(p) = p % 32
        for h in range(2):
            sf_ps = pbig.tile([P, 1024], F32, tag="big")
            for n in range(2):
                for dk in range(2):
                    nm = h * 2 + n
                    nc.tensor.matmul(
                        out=sf_ps[:, n * 512:(n + 1) * 512],
                        lhsT=cuT_sb[:, dk * 128:(dk + 1) * 128],
                        rhs=w_sb[:, dk * M + nm * 512: dk * M + (nm + 1) * 512],
                        start=(dk == 0), stop=(dk == 1))
            sf_sb = work.tile([P, 1024], F32, tag="sf")
            nc.vector.tensor_copy(out=sf_sb[:, 0:512], in_=sf_ps[:, 0:512])
            nc.scalar.copy(out=sf_sb[:, 512:1024], in_=sf_ps[:, 512:1024])
            nc.sync.dma_start(out=sf_d[:, h * 1024:(h + 1) * 1024],
                              in_=sf_sb[:])
        s_sb = work.tile([P, DC], F32, tag="s")
        for bl in range(BL):
            src = bass.AP(tensor=sf_d.tensor, offset=bl * 32 * M,
                          ap=[[M + DC, 32], [1, DC]])
            nc.sync.dma_start(out=s_sb[bl * 32:(bl + 1) * 32, :], in_=src)

        # squash scale: rv = 1/sqrt(sum(s^2) + eps)
        sq_sb = work.tile([P, DC], F32, tag="sq")
        nc.vector.tensor_mul(sq_sb[:], s_sb[:], s_sb[:])
        ssq = work.tile([P, 1], F32, tag="ssq")
        nc.vector.reduce_sum(out=ssq[:], in_=sq_sb[:], axis=mybir.AxisListType.X)
        sr = work.tile([P, 1], F32, tag="sr")
        nc.scalar.activation(out=sr[:], in_=ssq[:], func=AF.Sqrt,
                             bias=eps_sb[:])
        rv = work.tile([P, 1], F32, tag="rv")
        nc.vector.reciprocal(out=rv[:], in_=sr[:])

        if r == ROUTINGS - 1:
            v_sb = work.tile([P, DC], F32, tag="v")
            nc.vector.tensor_scalar(out=v_sb[:], in0=s_sb[:],
                                    scalar1=rv[:, 0:1], scalar2=None,
                                    op0=ALU.mult)
            nc.sync.dma_start(out=v_d[:], in_=v_sb[:])
            continue

        # s2 = [s, s] duplicated along free dim; s2T[t*64+d', p] = s[p, d']
        s2_sb = work.tile([P, 2 * DC], F32, tag="s2")
        nc.scalar.copy(out=s2_sb[:, 0:DC], in_=s_sb[:])
        nc.scalar.copy(out=s2_sb[:, DC:2 * DC], in_=s_sb[:])
        pt_f = ptr_f.tile([P, P], F32, tag="trf")
        nc.tensor.transpose(out=pt_f[:], in_=s2_sb[:], identity=ident[:])
        s2T_sb = work.tile([P, P], F32, tag="s2T")
        nc.vector.tensor_copy(out=s2T_sb[:], in_=pt_f[:])

        # scatter s into the block-diagonal embedding vemb[m_tile k]:
        # vemb_k[t*64+d', p] = s[p, d'] for p with capsule i(p) == 2k+t
        for k in range(16):
            for t in range(2):
                i_cap = 2 * k + t
                rows = slice(t * 64, (t + 1) * 64)
                src = s2T_sb[rows, :].rearrange("p (b c) -> p b c", b=4)[:, :, i_cap]
                dst = vemb[rows, k * P:(k + 1) * P].rearrange(
                    "p (b c) -> p b c", b=4)[:, :, i_cap]
                nc.vector.tensor_copy(out=dst, in_=src)

        # w_v[p, d] = sum_{d'} s[p, d'] * W[d, i(p)*64+d']  (then scaled by rv)
        wv_ps = pmm.tile([P, D], F32, tag="mm")
        for k in range(16):
            nc.tensor.matmul(out=wv_ps[:],
                             lhsT=vemb[:, k * P:(k + 1) * P],
                             rhs=wT_sb[:, k * D:(k + 1) * D],
                             start=(k == 0), stop=(k == 15))
        wv_sb = work.tile([P, D], F32R, tag="wv")
        nc.vector.tensor_scalar(out=wv_sb[:], in0=wv_ps[:],
                                scalar1=rv[:, 0:1], scalar2=None, op0=ALU.mult)
        # transpose wv and scatter into block-masked wvm tiles
        for dk in range(2):
            pt = ptr_r.tile([P, P], F32R, tag="trr")
            nc.tensor.transpose(out=pt[:], in_=wv_sb[:, dk * 128:(dk + 1) * 128],
                                identity=ident_r[:])
            for bl in range(BL):
                cols = slice(bl * 32, bl * 32 + 32)
                copy_engines[bl % 2](
                    out=wvm[:, (bl * 2 + dk) * P:(bl * 2 + dk) * P + P][:, cols],
                    in_=pt[:, cols])

        # bu[p, j] = sum_d wv[p, d] * u[bl(p)][j, d] as one accumulation over
        # the concatenated (bl, dk) axis with block-masked weights
        bu_ps = pbig.tile([P, N], F32, tag="big")
        for n in range(2):
            for bl in range(BL):
                for dk in range(2):
                    nc.tensor.matmul(
                        out=bu_ps[:, n * 512:(n + 1) * 512],
                        lhsT=wvm[:, (bl * 2 + dk) * P:(bl * 2 + dk + 1) * P],
                        rhs=uT_sb[:, (bl * 2 + dk) * N + n * 512:
                                  (bl * 2 + dk) * N + (n + 1) * 512],
                        start=(bl == 0 and dk == 0), stop=(bl == 3 and dk == 1))

        # b += bu ; eTm[(bl,jk)] = masked exp(b)^T (exp fused into copy-out)
        if r == 0:
            nc.vector.tensor_copy(out=b_sb[:], in_=bu_ps[:])
        else:
            nc.vector.tensor_add(out=b_sb[:], in0=b_sb[:], in1=bu_ps[:])
        for jk in range(8):
            pt_f = ptr_f.tile([P, P], F32, tag="trf")
            nc.tensor.transpose(out=pt_f[:], in_=b_sb[:, jk * 128:(jk + 1) * 128],
                                identity=ident[:])
            for bl in range(BL):
                cols = slice(bl * 32, bl * 32 + 32)
                nc.scalar.activation(
                    out=eTm[:, (bl * 8 + jk) * P:(bl * 8 + jk + 1) * P][:, cols],
                    in_=pt_f[:, cols], func=AF.Exp)


_NC_CACHE = None


def _get_nc():
    global _NC_CACHE
    if _NC_CACHE is None:
        _NC_CACHE = _build_kernel()
    return _NC_CACHE


def kernel(u_vecs: np.ndarray, W: np.ndarray) -> np.ndarray:
    u_vecs = np.ascontiguousarray(np.asarray(u_vecs, dtype=np.float32))
    W0 = np.ascontiguousarray(np.asarray(W, dtype=np.float32).reshape(D, M))
    nc = _get_nc()
    in_maps = [
        {"u": u_vecs[c * BL:(c + 1) * BL].reshape(BL * N, D), "w": W0}
        for c in range(N_CORES)
    ]
    res = bass_utils.run_bass_kernel_spmd(nc, in_maps,
                                          core_ids=list(range(N_CORES)))
    out = np.empty((B, NC, DC), dtype=np.float32)
    for c in range(N_CORES):
        out[c * BL:(c + 1) * BL] = res.results[c]["v"].reshape(BL, NC, DC)
    return out


# revision 11
# speedup vs baseline: 1.5838x; 1.0744x over previous
"""Trainium2 Bass/Tile kernel for nn_Capsule_6004364280312.

Computes CapsNet dynamic routing:
    u_hat = einsum('bnd,dm->bnm', u_vecs, W[0]) reshaped to [B, NC, N, DC]
    3 rounds of routing (softmax over N / weighted sum / squash / agreement)
    returns v [B, NC, DC]

Strategy (per core, batch-parallel over 8 cores, 4 batches each):
  * never materialize u_hat (268 MB). Algebra:
        s[i]  = (e[i] @ u) @ W_i          (e = exp(b), unnormalized softmax)
        b[i] += u @ (W_i @ (s[i] * rsqrt(||s[i]||^2 + eps)))
    The softmax normalizer cancels: v = normalize(s) is invariant to row
    scaling of e, so softmax is just exp().
  * partition layout p = bl*32 + i  (bl = local batch 0..3, i = capsule 0..31)
    so per-round tensors are full-width [128, *].
  * all matmul operands in float32r (TF32-like, 4x faster PE than fp32;
    measured rel err ~1.5e-4 per matmul, final ~3e-4 << resid_var gate).
    f32r matmuls require dst partition base 0, so the per-batch (cu) and
    per-batch (bu) contractions use block-masked weights over the
    concatenated contraction axis instead of col-tiled partial outputs.
  * block-diagonal extraction of s from the full [128, 2048] product via a
    small DRAM bounce with a strided (diagonal) access pattern.
"""

import numpy as np
from contextlib import ExitStack

import concourse.bass as bass
import concourse.mybir as mybir
import concourse.tile as tile
from concourse import bacc, bass_utils
from concourse.masks import make_identity

F32 = mybir.dt.float32
F32R = mybir.dt.float32r
AF = mybir.ActivationFunctionType
ALU = mybir.AluOpType

B, N, D = 32, 1024, 256
NC, DC = 32, 64
M = NC * DC  # 2048
N_CORES = 8
BL = B // N_CORES  # local batches per core
P = 128
EPS = 1e-7
ROUTINGS = 3


def _build_kernel():
    nc = bacc.Bacc("TRN2", target_bir_lowering=False, debug=False,
                   num_devices=N_CORES)
    u_d = nc.dram_tensor("u", (BL * N, D), F32, kind="ExternalInput").ap()
    w_d = nc.dram_tensor("w", (D, M), F32, kind="ExternalInput").ap()
    v_d = nc.dram_tensor("v", (P, DC), F32, kind="ExternalOutput").ap()
    sf_d = nc.dram_tensor("sf_scratch", (P, M), F32, kind="Internal").ap()

    with tile.TileContext(nc) as tc:
        with ExitStack() as ctx:
            _body(ctx, tc, v_d, u_d, w_d, sf_d)
    nc.compile()
    return nc


def _body(ctx, tc, v_d, u_d, w_d, sf_d):
    nc = tc.nc
    const = ctx.enter_context(tc.tile_pool(name="const", bufs=1))
    work = ctx.enter_context(tc.tile_pool(name="work", bufs=2))
    ptr_r = ctx.enter_context(tc.tile_pool(name="ptr_r", bufs=2, space="PSUM"))
    ptr_f = ctx.enter_context(tc.tile_pool(name="ptr_f", bufs=2, space="PSUM"))
    pmm = ctx.enter_context(tc.tile_pool(name="pmm", bufs=2, space="PSUM"))
    pbig = ctx.enter_context(tc.tile_pool(name="pbig", bufs=1, space="PSUM"))

    # ---------------- persistent SBUF state ----------------
    ident = const.tile([P, P], F32)
    make_identity(nc, ident)
    ident_r = const.tile([P, P], F32R)
    nc.vector.tensor_copy(out=ident_r[:], in_=ident[:])
    eps_sb = const.tile([P, 1], F32)
    nc.vector.memset(eps_sb, EPS)

    # block-masked all-ones weights for round 0 (uniform softmax):
    # onesm[bl] = [128, 128] with cols [32bl, 32bl+32) = 1, else 0
    onesm = const.tile([P, BL * P], F32R)
    nc.vector.memset(onesm[:].bitcast(F32), 0.0)
    for bl in range(BL):
        nc.vector.memset(
            onesm[:, bl * P + bl * 32: bl * P + bl * 32 + 32].bitcast(F32), 1.0)

    u_sb = const.tile([P, BL * 8 * D], F32R)   # u[bl][jk]: [128(j), 256(d)]
    uT_sb = const.tile([P, BL * 2 * N], F32R)  # uT[bl][dk]: [128(d), 1024(j)]
    w_sb = const.tile([P, 2 * M], F32R)        # w[dk]: [128(d), 2048(m)]
    wT_sb = const.tile([P, 16 * D], F32R)      # wT[mk]: [128(m), 256(d)]
    b_sb = const.tile([P, N], F32)             # routing logits
    # block-masked exp(b)^T: eTm[(bl,jk)][j_local, p] = e[p, jk*128+j_local]
    # for p in bl's block, else 0
    eTm = const.tile([P, BL * 8 * P], F32R)
    nc.vector.memset(eTm[:].bitcast(F32), 0.0)
    # block-masked wv^T: wvm[(bl,dk)][d_local, p] masked to bl's block
    wvm = const.tile([P, BL * 2 * P], F32R)
    nc.vector.memset(wvm[:].bitcast(F32), 0.0)
    vemb = const.tile([P, 16 * P], F32R)       # block-diag s embedding
    nc.vector.memset(vemb[:].bitcast(F32), 0.0)

    copy_engines = [nc.scalar.copy, nc.vector.tensor_copy]

    def tr_r(dst_ap, src_ap, eng):
        """f32r 128x128 transpose src -> psum -> dst."""
        pt = ptr_r.tile([P, P], F32R, tag="trr")
        nc.tensor.transpose(out=pt[:], in_=src_ap, identity=ident_r[:])
        copy_engines[eng % 2](out=dst_ap, in_=pt[:])

    def tr_f2r(dst_ap, src_ap, eng):
        """f32 input 128x128 transpose -> psum f32 -> cast-copy to f32r dst."""
        pt = ptr_f.tile([P, P], F32, tag="trf")
        nc.tensor.transpose(out=pt[:], in_=src_ap, identity=ident[:])
        copy_engines[eng % 2](out=dst_ap, in_=pt[:])

    # setup staging (f32 loads on fast HWDGE; engine copies round to f32r)
    stage = ctx.enter_context(tc.tile_pool(name="stage", bufs=4))

    # ---------------- load W + build W^T ----------------
    for dk in range(2):
        wst = stage.tile([P, M], F32, tag="wst")
        nc.sync.dma_start(out=wst[:], in_=w_d[dk * 128:(dk + 1) * 128, :])
        for half in range(2):
            copy_engines[half](
                out=w_sb[:, dk * M + half * 1024: dk * M + (half + 1) * 1024],
                in_=wst[:, half * 1024:(half + 1) * 1024])
        for mk in range(16):
            tr_f2r(wT_sb[:, mk * D + dk * 128: mk * D + (dk + 1) * 128],
                   wst[:, mk * 128:(mk + 1) * 128], eng=mk)

    # ---------------- load u + build u^T ----------------
    for bl in range(BL):
        for jk in range(8):
            off = (bl * 8 + jk) * D
            ust = stage.tile([P, D], F32, tag="ust")
            nc.sync.dma_start(
                out=ust[:],
                in_=u_d[bl * N + jk * 128: bl * N + (jk + 1) * 128, :])
            copy_engines[jk % 2](out=u_sb[:, off:off + D], in_=ust[:])
            for dk in range(2):
                tr_f2r(uT_sb[:, (bl * 2 + dk) * N + jk * 128:
                             (bl * 2 + dk) * N + (jk + 1) * 128],
                       ust[:, dk * 128:(dk + 1) * 128],
                       eng=jk * 2 + dk + 1)

    # ---------------- routing rounds ----------------
    for r in range(ROUTINGS):
        # cu[p, d] = sum_j e[p, j] * u[bl(p)][j, d] as one accumulation over
        # the concatenated (bl, jk) axis with block-masked weights
        cu_ps = pmm.tile([P, D], F32, tag="mm")
        first, last = (0, 0), (BL - 1, 7)
        for bl in range(BL):
            for jk in range(8):
                lhs = (onesm[:, bl * P:(bl + 1) * P] if r == 0 else
                       eTm[:, (bl * 8 + jk) * P:(bl * 8 + jk + 1) * P])
                nc.tensor.matmul(
                    out=cu_ps[:],
                    lhsT=lhs,
                    rhs=u_sb[:, (bl * 8 + jk) * D:(bl * 8 + jk + 1) * D],
                    start=((bl, jk) == first), stop=((bl, jk) == last))
        cu_sb = work.tile([P, D], F32R, tag="cu")
        nc.scalar.copy(out=cu_sb[:, 0:128], in_=cu_ps[:, 0:128])
        nc.vector.tensor_copy(out=cu_sb[:, 128:256], in_=cu_ps[:, 128:256])
        cuT_sb = work.tile([P, D], F32R, tag="cuT")
        for dk in range(2):
            tr_r(cuT_sb[:, dk * 128:(dk + 1) * 128],
                 cu_sb[:, dk * 128:(dk + 1) * 128], eng=dk)

        # s_full[p, m] = sum_d cu[p, d] * W[d, m]; two psum halves,
        # bounce each through DRAM to extract diagonal blocks:
        # s[p, d'] = s_full[p, i(p)*64 + d'],  i(p) = p % 32
        # (half h holds capsules [16h, 16h+16), so extract per half)
        s_sb = work.tile([P, DC], F32, tag="s")
        for h in range(2):
            sf_ps = pbig.tile([P, 1024], F32, tag="big")
            for n in range(2):
                for dk in range(2):
                    nm = h * 2 + n
                    nc.tensor.matmul(
                        out=sf_ps[:, n * 512:(n + 1) * 512],
                        lhsT=cuT_sb[:, dk * 128:(dk + 1) * 128],
                        rhs=w_sb[:, dk * M + nm * 512: dk * M + (nm + 1) * 512],
                        start=(dk == 0), stop=(dk == 1))
            sf_sb = work.tile([P, 1024], F32, tag="sf")
            for q in range(2):
                copy_engines[q](out=sf_sb[:, q * 512:(q + 1) * 512],
                                in_=sf_ps[:, q * 512:(q + 1) * 512])
                nc.sync.dma_start(
                    out=sf_d[:, h * 1024 + q * 512: h * 1024 + (q + 1) * 512],
                    in_=sf_sb[:, q * 512:(q + 1) * 512])
            for bl in range(BL):
                src = bass.AP(tensor=sf_d.tensor,
                              offset=bl * 32 * M + 16 * h * (M + DC),
                              ap=[[M + DC, 16], [1, DC]])
                nc.sync.dma_start(
                    out=s_sb[bl * 32 + 16 * h: bl * 32 + 16 * h + 16, :],
                    in_=src)

        # squash scale: rv = 1/sqrt(sum(s^2) + eps)
        sq_sb = work.tile([P, DC], F32, tag="sq")
        nc.vector.tensor_mul(sq_sb[:], s_sb[:], s_sb[:])
        ssq = work.tile([P, 1], F32, tag="ssq")
        nc.vector.reduce_sum(out=ssq[:], in_=sq_sb[:], axis=mybir.AxisListType.X)
        sr = work.tile([P, 1], F32, tag="sr")
        nc.scalar.activation(out=sr[:], in_=ssq[:], func=AF.Sqrt,
                             bias=eps_sb[:])
        rv = work.tile([P, 1], F32, tag="rv")
        nc.vector.reciprocal(out=rv[:], in_=sr[:])

        if r == ROUTINGS - 1:
            v_sb = work.tile([P, DC], F32, tag="v")
            nc.vector.tensor_scalar(out=v_sb[:], in0=s_sb[:],
                                    scalar1=rv[:, 0:1], scalar2=None,
                                    op0=ALU.mult)
            nc.sync.dma_start(out=v_d[:], in_=v_sb[:])
            continue

        # s2 = [s, s] duplicated along free dim; s2T[t*64+d', p] = s[p, d']
        s2_sb = work.tile([P, 2 * DC], F32, tag="s2")
        nc.scalar.copy(out=s2_sb[:, 0:DC], in_=s_sb[:])
        nc.scalar.copy(out=s2_sb[:, DC:2 * DC], in_=s_sb[:])
        pt_f = ptr_f.tile([P, P], F32, tag="trf")
        nc.tensor.transpose(out=pt_f[:], in_=s2_sb[:], identity=ident[:])
        s2T_sb = work.tile([P, P], F32, tag="s2T")
        nc.vector.tensor_copy(out=s2T_sb[:], in_=pt_f[:])

        # scatter s into the block-diagonal embedding vemb[m_tile k]:
        # vemb_k[t*64+d', p] = s[p, d'] for p with capsule i(p) == 2k+t
        for k in range(16):
            for t in range(2):
                i_cap = 2 * k + t
                rows = slice(t * 64, (t + 1) * 64)
                src = s2T_sb[rows, :].rearrange("p (b c) -> p b c", b=4)[:, :, i_cap]
                dst = vemb[rows, k * P:(k + 1) * P].rearrange(
                    "p (b c) -> p b c", b=4)[:, :, i_cap]
                copy_engines[(2 * k + t) % 2](out=dst, in_=src)

        # w_v[p, d] = sum_{d'} s[p, d'] * W[d, i(p)*64+d']  (then scaled by rv)
        wv_ps = pmm.tile([P, D], F32, tag="mm")
        for k in range(16):
            nc.tensor.matmul(out=wv_ps[:],
                             lhsT=vemb[:, k * P:(k + 1) * P],
                             rhs=wT_sb[:, k * D:(k + 1) * D],
                             start=(k == 0), stop=(k == 15))
        wv_sb = work.tile([P, D], F32R, tag="wv")
        nc.vector.tensor_scalar(out=wv_sb[:], in0=wv_ps[:],
                                scalar1=rv[:, 0:1], scalar2=None, op0=ALU.mult)
        # transpose wv and scatter into block-masked wvm tiles
        for dk in range(2):
            pt = ptr_r.tile([P, P], F32R, tag="trr")
            nc.tensor.transpose(out=pt[:], in_=wv_sb[:, dk * 128:(dk + 1) * 128],
                                identity=ident_r[:])
            for bl in range(BL):
                cols = slice(bl * 32, bl * 32 + 32)
                copy_engines[bl % 2](
                    out=wvm[:, (bl * 2 + dk) * P:(bl * 2 + dk) * P + P][:, cols],
                    in_=pt[:, cols])

        # bu[p, j] = sum_d wv[p, d] * u[bl(p)][j, d] as one accumulation over
        # the concatenated (bl, dk) axis with block-masked weights
        bu_ps = pbig.tile([P, N], F32, tag="big")
        for n in range(2):
            for bl in range(BL):
                for dk in range(2):
                    nc.tensor.matmul(
                        out=bu_ps[:, n * 512:(n + 1) * 512],
                        lhsT=wvm[:, (bl * 2 + dk) * P:(bl * 2 + dk + 1) * P],
                        rhs=uT_sb[:, (bl * 2 + dk) * N + n * 512:
                                  (bl * 2 + dk) * N + (n + 1) * 512],
                        start=(bl == 0 and dk == 0), stop=(bl == 3 and dk == 1))

        # b += bu ; eTm[(bl,jk)] = masked exp(b)^T (exp fused into copy-out);
        # chunked so each transpose can start as soon as its chunk is added
        for jk in range(8):
            sl = slice(jk * 128, (jk + 1) * 128)
            if r == 0:
                nc.vector.tensor_copy(out=b_sb[:, sl], in_=bu_ps[:, sl])
            else:
                nc.vector.tensor_add(out=b_sb[:, sl], in0=b_sb[:, sl],
                                     in1=bu_ps[:, sl])
            pt_f = ptr_f.tile([P, P], F32, tag="trf")
            nc.tensor.transpose(out=pt_f[:], in_=b_sb[:, sl],
                                identity=ident[:])
            for bl in range(BL):
                cols = slice(bl * 32, bl * 32 + 32)
                nc.scalar.activation(
                    out=eTm[:, (bl * 8 + jk) * P:(bl * 8 + jk + 1) * P][:, cols],
                    in_=pt_f[:, cols], func=AF.Exp)


_NC_CACHE = None


def _get_nc():
    global _NC_CACHE
    if _NC_CACHE is None:
        _NC_CACHE = _build_kernel()
    return _NC_CACHE


def kernel(u_vecs: np.ndarray, W: np.ndarray) -> np.ndarray:
    u_vecs = np.ascontiguousarray(np.asarray(u_vecs, dtype=np.float32))
    W0 = np.ascontiguousarray(np.asarray(W, dtype=np.float32).reshape(D, M))
    nc = _get_nc()
    in_maps = [
        {"u": u_vecs[c * BL:(c + 1) * BL].reshape(BL * N, D), "w": W0}
        for c in range(N_CORES)
    ]
    res = bass_utils.run_bass_kernel_spmd(nc, in_maps,
                                          core_ids=list(range(N_CORES)))
    out = np.empty((B, NC, DC), dtype=np.float32)
    for c in range(N_CORES):
        out[c * BL:(c + 1) * BL] = res.results[c]["v"].reshape(BL, NC, DC)
    return out


# revision 15
# speedup vs baseline: 1.7441x; 1.1012x over previous
"""Trainium2 Bass/Tile kernel for nn_Capsule_6004364280312.

Computes CapsNet dynamic routing:
    u_hat = einsum('bnd,dm->bnm', u_vecs, W[0]) reshaped to [B, NC, N, DC]
    3 rounds of routing (softmax over N / weighted sum / squash / agreement)
    returns v [B, NC, DC]

Strategy (per core, batch-parallel over 8 cores, 4 batches each):
  * never materialize u_hat (268 MB). Algebra:
        s[i]  = (e[i] @ u) @ W_i          (e = exp(b), unnormalized softmax)
        b[i] += u @ (W_i @ (s[i] * rsqrt(||s[i]||^2 + eps)))
    The softmax normalizer cancels: v = normalize(s) is invariant to row
    scaling of e, so softmax is just exp().
  * partition layout p = bl*32 + i  (bl = local batch 0..3, i = capsule 0..31)
    so per-round tensors are full-width [128, *].
  * all matmul operands in float32r (TF32-like, 4x faster PE than fp32;
    measured rel err ~1.5e-4 per matmul, final ~3e-4, resid_var ~1e-7).
    f32r matmuls require dst partition base 0, so the per-batch (cu/bu)
    contractions run over the concatenated contraction axis with
    block-masked weights.
  * block-diagonal extraction of s from the full [128, 2048] product via a
    DRAM bounce with strided (diagonal) access patterns.
  * scatter/masked writes are single strided-AP ops; DMA count is minimized
    (the DMA queue engine costs ~620ns per dma_start).
"""

import numpy as np
from contextlib import ExitStack

import concourse.bass as bass
import concourse.mybir as mybir
import concourse.tile as tile
from concourse import bacc, bass_utils
from concourse.masks import make_identity

F32 = mybir.dt.float32
F32R = mybir.dt.float32r
AF = mybir.ActivationFunctionType
ALU = mybir.AluOpType

B, N, D = 32, 1024, 256
NC, DC = 32, 64
M = NC * DC  # 2048
N_CORES = 8
BL = B // N_CORES  # local batches per core
P = 128
EPS = 1e-7
ROUTINGS = 3


def _ap(base, offset, dims):
    """Raw strided AP over the same tensor as `base` (flat element space)."""
    return bass.AP(tensor=base.tensor, offset=offset, ap=dims)


def _build_kernel():
    nc = bacc.Bacc("TRN2", target_bir_lowering=False, debug=False,
                   num_devices=N_CORES)
    u_d = nc.dram_tensor("u", (BL * N, D), F32, kind="ExternalInput").ap()
    w_d = nc.dram_tensor("w", (D, M), F32, kind="ExternalInput").ap()
    v_d = nc.dram_tensor("v", (P, DC), F32, kind="ExternalOutput").ap()
    sf_d = nc.dram_tensor("sf_scratch", (P, M), F32, kind="Internal").ap()

    with tile.TileContext(nc) as tc:
        with ExitStack() as ctx:
            _body(ctx, tc, v_d, u_d, w_d, sf_d)
    nc.compile()
    return nc


def _body(ctx, tc, v_d, u_d, w_d, sf_d):
    nc = tc.nc
    const = ctx.enter_context(tc.tile_pool(name="const", bufs=1))
    work = ctx.enter_context(tc.tile_pool(name="work", bufs=2))
    stage = ctx.enter_context(tc.tile_pool(name="stage", bufs=2))
    ptr_r = ctx.enter_context(tc.tile_pool(name="ptr_r", bufs=2, space="PSUM"))
    ptr_f = ctx.enter_context(tc.tile_pool(name="ptr_f", bufs=2, space="PSUM"))
    pmm = ctx.enter_context(tc.tile_pool(name="pmm", bufs=2, space="PSUM"))
    pbig = ctx.enter_context(tc.tile_pool(name="pbig", bufs=1, space="PSUM"))

    # ---------------- persistent SBUF state ----------------
    ident = const.tile([P, P], F32)
    make_identity(nc, ident)
    ident_r = const.tile([P, P], F32R)
    nc.gpsimd.tensor_copy(out=ident_r[:], in_=ident[:])
    eps_sb = const.tile([P, 1], F32)
    nc.gpsimd.memset(eps_sb[:].bitcast(F32), EPS)

    # block-masked all-ones weights for round 0 (uniform softmax):
    # onesm[bl] = [128, 128] with cols [32bl, 32bl+32) = 1, else 0
    onesm = const.tile([P, BL * P], F32R)
    nc.gpsimd.memset(onesm[:].bitcast(F32), 0.0)
    nc.gpsimd.memset(
        _ap(onesm[:], 0, [[BL * P, P], [P + 32, BL], [1, 32]]).bitcast(F32), 1.0)

    u_sb = const.tile([P, BL * 8 * D], F32R)   # u[bl][jk]: [128(j), 256(d)]
    uT_sb = const.tile([P, BL * 2 * N], F32R)  # uT[bl][dk]: [128(d), 1024(j)]
    w_sb = const.tile([P, 2 * M], F32R)        # w[dk]: [128(d), 2048(m)]
    wT_sb = const.tile([P, 16 * D], F32R)      # wT[mk]: [128(m), 256(d)]
    b_sb = const.tile([P, N], F32)             # routing logits
    # block-masked exp(b)^T: eTm[(bl,jk)][j_local, p] = e[p, jk*128+j_local]
    # for p in bl's block, else 0
    eTm = const.tile([P, BL * 8 * P], F32R)
    nc.gpsimd.memset(eTm[:].bitcast(F32), 0.0)
    # block-masked wv^T: wvm[(bl,dk)][d_local, p] masked to bl's block
    wvm = const.tile([P, BL * 2 * P], F32R)
    nc.gpsimd.memset(wvm[:].bitcast(F32), 0.0)
    vemb = const.tile([P, 16 * P], F32R)       # block-diag s embedding
    nc.gpsimd.memset(vemb[:].bitcast(F32), 0.0)

    copy_engines = [nc.scalar.copy, nc.vector.tensor_copy]

    def tr_r(dst_ap, src_ap, eng):
        """f32r 128x128 transpose src -> psum -> dst."""
        pt = ptr_r.tile([P, P], F32R, tag="trr")
        nc.tensor.transpose(out=pt[:], in_=src_ap, identity=ident_r[:])
        copy_engines[eng % 2](out=dst_ap, in_=pt[:])

    def tr_f2r(dst_ap, src_ap, eng):
        """f32 input 128x128 transpose -> psum f32 -> cast-copy to f32r dst."""
        pt = ptr_f.tile([P, P], F32, tag="trf")
        nc.tensor.transpose(out=pt[:], in_=src_ap, identity=ident[:])
        copy_engines[eng % 2](out=dst_ap, in_=pt[:])

    # ---------------- load W + build W^T ----------------
    for dk in range(2):
        wst = stage.tile([P, M], F32, tag="wst")
        nc.sync.dma_start(out=wst[:], in_=w_d[dk * 128:(dk + 1) * 128, :])
        for half in range(2):
            copy_engines[half](
                out=w_sb[:, dk * M + half * 1024: dk * M + (half + 1) * 1024],
                in_=wst[:, half * 1024:(half + 1) * 1024])
        for mk in range(16):
            tr_f2r(wT_sb[:, mk * D + dk * 128: mk * D + (dk + 1) * 128],
                   wst[:, mk * 128:(mk + 1) * 128], eng=mk)

    # ---------------- load u (one DMA per local batch) + build u^T --------
    for bl in range(BL):
        ust = stage.tile([P, 8 * D], F32, tag="ust")
        # gather the 8 j-tiles of batch bl in one DMA:
        # dst[p, (jk, d)] = u[bl*1024 + jk*128 + p, d]
        src = _ap(u_d, bl * N * D, [[D, P], [P * D, 8], [1, D]])
        nc.sync.dma_start(out=ust[:].rearrange("p (jk d) -> p jk d", jk=8),
                          in_=src)
        for half in range(2):
            copy_engines[half](
                out=u_sb[:, bl * 8 * D + half * 1024:
                         bl * 8 * D + (half + 1) * 1024],
                in_=ust[:, half * 1024:(half + 1) * 1024])
        for jk in range(8):
            for dk in range(2):
                tr_f2r(uT_sb[:, (bl * 2 + dk) * N + jk * 128:
                             (bl * 2 + dk) * N + (jk + 1) * 128],
                       ust[:, jk * D + dk * 128: jk * D + (dk + 1) * 128],
                       eng=jk * 2 + dk + 1)

    # ---------------- routing rounds ----------------
    for r in range(ROUTINGS):
        # cu[p, d] = sum_j e[p, j] * u[bl(p)][j, d] as one accumulation over
        # the concatenated (bl, jk) axis with block-masked weights
        cu_ps = pmm.tile([P, D], F32, tag="mm")
        first, last = (0, 0), (BL - 1, 7)
        for bl in range(BL):
            for jk in range(8):
                lhs = (onesm[:, bl * P:(bl + 1) * P] if r == 0 else
                       eTm[:, (bl * 8 + jk) * P:(bl * 8 + jk + 1) * P])
                nc.tensor.matmul(
                    out=cu_ps[:],
                    lhsT=lhs,
                    rhs=u_sb[:, (bl * 8 + jk) * D:(bl * 8 + jk + 1) * D],
                    start=((bl, jk) == first), stop=((bl, jk) == last))
        cu_sb = work.tile([P, D], F32R, tag="cu")
        nc.scalar.copy(out=cu_sb[:, 0:128], in_=cu_ps[:, 0:128])
        nc.vector.tensor_copy(out=cu_sb[:, 128:256], in_=cu_ps[:, 128:256])
        cuT_sb = work.tile([P, D], F32R, tag="cuT")
        for dk in range(2):
            tr_r(cuT_sb[:, dk * 128:(dk + 1) * 128],
                 cu_sb[:, dk * 128:(dk + 1) * 128], eng=dk)

        # s_full[p, m] = sum_d cu[p, d] * W[d, m]; two psum halves,
        # bounce each through DRAM to extract diagonal blocks:
        # s[p, d'] = s_full[p, i(p)*64 + d'],  i(p) = p % 32
        # (half h holds capsules [16h, 16h+16), so extract per half)
        s_sb = work.tile([P, DC], F32, tag="s")
        for h in range(2):
            sf_ps = pbig.tile([P, 1024], F32, tag="big")
            for n in range(2):
                for dk in range(2):
                    nm = h * 2 + n
                    nc.tensor.matmul(
                        out=sf_ps[:, n * 512:(n + 1) * 512],
                        lhsT=cuT_sb[:, dk * 128:(dk + 1) * 128],
                        rhs=w_sb[:, dk * M + nm * 512: dk * M + (nm + 1) * 512],
                        start=(dk == 0), stop=(dk == 1))
            sf_sb = work.tile([P, 1024], F32, tag="sf")
            for q in range(2):
                copy_engines[q](out=sf_sb[:, q * 512:(q + 1) * 512],
                                in_=sf_ps[:, q * 512:(q + 1) * 512])
            nc.sync.dma_start(out=sf_d[:, h * 1024:(h + 1) * 1024],
                              in_=sf_sb[:])
        for bl in range(BL):
            src = _ap(sf_d, bl * 32 * M, [[M + DC, 32], [1, DC]])
            nc.sync.dma_start(out=s_sb[bl * 32:(bl + 1) * 32, :], in_=src)

        # squash scale: rv = 1/sqrt(sum(s^2) + eps)
        sq_sb = work.tile([P, DC], F32, tag="sq")
        nc.vector.tensor_mul(sq_sb[:], s_sb[:], s_sb[:])
        ssq = work.tile([P, 1], F32, tag="ssq")
        nc.vector.reduce_sum(out=ssq[:], in_=sq_sb[:], axis=mybir.AxisListType.X)
        sr = work.tile([P, 1], F32, tag="sr")
        nc.scalar.activation(out=sr[:], in_=ssq[:], func=AF.Sqrt,
                             bias=eps_sb[:])
        rv = work.tile([P, 1], F32, tag="rv")
        nc.vector.reciprocal(out=rv[:], in_=sr[:])

        if r == ROUTINGS - 1:
            v_sb = work.tile([P, DC], F32, tag="v")
            nc.vector.tensor_scalar(out=v_sb[:], in0=s_sb[:],
                                    scalar1=rv[:, 0:1], scalar2=None,
                                    op0=ALU.mult)
            nc.sync.dma_start(out=v_d[:], in_=v_sb[:])
            continue

        # s2 = [s, s] duplicated along free dim; s2T[t*64+d', p] = s[p, d']
        s2_sb = work.tile([P, 2 * DC], F32, tag="s2")
        nc.scalar.copy(out=s2_sb[:].rearrange("p (t c) -> p t c", t=2),
                       in_=s_sb[:].unsqueeze(1).to_broadcast([P, 2, DC]))
        pt_f = ptr_f.tile([P, P], F32, tag="trf")
        nc.tensor.transpose(out=pt_f[:], in_=s2_sb[:], identity=ident[:])
        s2T_sb = work.tile([P, P], F32, tag="s2T")
        nc.vector.tensor_copy(out=s2T_sb[:], in_=pt_f[:])

        # scatter s into the block-diagonal embedding vemb[m_tile k]:
        # vemb_k[t*64+d', p] = s[p, d'] for p with capsule i(p) == 2k+t
        # one strided-AP copy per t
        for t in range(2):
            src = _ap(s2T_sb[:], t * 64 * P + t, [[P, 64], [2, 16], [32, 4]])
            dst = _ap(vemb[:], t * 64 * (16 * P) + t,
                      [[16 * P, 64], [P + 2, 16], [32, 4]])
            copy_engines[t](out=dst, in_=src)

        # w_v[p, d] = sum_{d'} s[p, d'] * W[d, i(p)*64+d']  (then scaled by rv)
        wv_ps = pmm.tile([P, D], F32, tag="mm")
        for k in range(16):
            nc.tensor.matmul(out=wv_ps[:],
                             lhsT=vemb[:, k * P:(k + 1) * P],
                             rhs=wT_sb[:, k * D:(k + 1) * D],
                             start=(k == 0), stop=(k == 15))
        wv_sb = work.tile([P, D], F32R, tag="wv")
        nc.vector.tensor_scalar(out=wv_sb[:], in0=wv_ps[:],
                                scalar1=rv[:, 0:1], scalar2=None, op0=ALU.mult)
        # transpose wv and scatter into block-masked wvm tiles (one op per dk)
        for dk in range(2):
            pt = ptr_r.tile([P, P], F32R, tag="trr")
            nc.tensor.transpose(out=pt[:], in_=wv_sb[:, dk * 128:(dk + 1) * 128],
                                identity=ident_r[:])
            dst = _ap(wvm[:], dk * P, [[BL * 2 * P, P], [2 * P + 32, BL], [1, 32]])
            copy_engines[dk](out=dst,
                             in_=pt[:].rearrange("p (b c) -> p b c", b=4))

        # bu[p, j] = sum_d wv[p, d] * u[bl(p)][j, d] as one accumulation over
        # the concatenated (bl, dk) axis with block-masked weights
        bu_ps = pbig.tile([P, N], F32, tag="big")
        for n in range(2):
            for bl in range(BL):
                for dk in range(2):
                    nc.tensor.matmul(
                        out=bu_ps[:, n * 512:(n + 1) * 512],
                        lhsT=wvm[:, (bl * 2 + dk) * P:(bl * 2 + dk + 1) * P],
                        rhs=uT_sb[:, (bl * 2 + dk) * N + n * 512:
                                  (bl * 2 + dk) * N + (n + 1) * 512],
                        start=(bl == 0 and dk == 0), stop=(bl == 3 and dk == 1))

        # b += bu ; eTm[(bl,jk)] = masked exp(b)^T (exp fused into copy-out,
        # one strided-AP activation per jk); chunked so each transpose can
        # start as soon as its chunk of b is updated
        for jk in range(8):
            sl = slice(jk * 128, (jk + 1) * 128)
            if r == 0:
                nc.vector.tensor_copy(out=b_sb[:, sl], in_=bu_ps[:, sl])
            else:
                nc.vector.tensor_add(out=b_sb[:, sl], in0=b_sb[:, sl],
                                     in1=bu_ps[:, sl])
            pt_f = ptr_f.tile([P, P], F32, tag="trf")
            nc.tensor.transpose(out=pt_f[:], in_=b_sb[:, sl],
                                identity=ident[:])
            dst = _ap(eTm[:], jk * P,
                      [[BL * 8 * P, P], [8 * P + 32, BL], [1, 32]])
            nc.scalar.activation(out=dst,
                                 in_=pt_f[:].rearrange("p (b c) -> p b c", b=4),
                                 func=AF.Exp)


_NC_CACHE = None


def _get_nc():
    global _NC_CACHE
    if _NC_CACHE is None:
        _NC_CACHE = _build_kernel()
    return _NC_CACHE


def kernel(u_vecs: np.ndarray, W: np.ndarray) -> np.ndarray:
    u_vecs = np.ascontiguousarray(np.asarray(u_vecs, dtype=np.float32))
    W0 = np.ascontiguousarray(np.asarray(W, dtype=np.float32).reshape(D, M))
    nc = _get_nc()
    in_maps = [
        {"u": u_vecs[c * BL:(c + 1) * BL].reshape(BL * N, D), "w": W0}
        for c in range(N_CORES)
    ]
    res = bass_utils.run_bass_kernel_spmd(nc, in_maps,
                                          core_ids=list(range(N_CORES)))
    out = np.empty((B, NC, DC), dtype=np.float32)
    for c in range(N_CORES):
        out[c * BL:(c + 1) * BL] = res.results[c]["v"].reshape(BL, NC, DC)
    return out


# revision 16
# speedup vs baseline: 1.9074x; 1.0936x over previous
"""Trainium2 Bass/Tile kernel for nn_Capsule_6004364280312.

Computes CapsNet dynamic routing:
    u_hat = einsum('bnd,dm->bnm', u_vecs, W[0]) reshaped to [B, NC, N, DC]
    3 rounds of routing (softmax over N / weighted sum / squash / agreement)
    returns v [B, NC, DC]

Strategy (per core, batch-parallel over 8 cores, 4 batches each):
  * never materialize u_hat (268 MB). Algebra:
        s[i]  = (e[i] @ u) @ W_i          (e = exp(b), unnormalized softmax)
        b[i] += u @ (W_i @ (s[i] * rsqrt(||s[i]||^2 + eps)))
    The softmax normalizer cancels: v = normalize(s) is invariant to row
    scaling of e, so softmax is just exp().
  * partition layout p = bl*32 + i  (bl = local batch 0..3, i = capsule 0..31)
    so per-round tensors are full-width [128, *].
  * all matmul operands in float32r (TF32-like, 4x faster PE than fp32;
    measured rel err ~1.5e-4 per matmul, final ~3e-4, resid_var ~1e-7).
    f32r matmuls require dst partition base 0, so the per-batch (cu/bu)
    contractions run over the concatenated contraction axis with
    block-masked weights.
  * block-diagonal extraction of s from the full [128, 2048] product via a
    DRAM bounce with strided (diagonal) access patterns.
  * scatter/masked writes are single strided-AP ops; DMA count is minimized
    (the DMA queue engine costs ~620ns per dma_start).
"""

import numpy as np
from contextlib import ExitStack

import concourse.bass as bass
import concourse.mybir as mybir
import concourse.tile as tile
from concourse import bacc, bass_utils
from concourse.masks import make_identity

F32 = mybir.dt.float32
F32R = mybir.dt.float32r
AF = mybir.ActivationFunctionType
ALU = mybir.AluOpType

B, N, D = 32, 1024, 256
NC, DC = 32, 64
M = NC * DC  # 2048
N_CORES = 8
BL = B // N_CORES  # local batches per core
P = 128
EPS = 1e-7
ROUTINGS = 3


def _ap(base, offset, dims):
    """Raw strided AP over the same tensor as `base` (flat element space)."""
    return bass.AP(tensor=base.tensor, offset=offset, ap=dims)


def _build_kernel():
    nc = bacc.Bacc("TRN2", target_bir_lowering=False, debug=False,
                   num_devices=N_CORES)
    u_d = nc.dram_tensor("u", (BL * N, D), F32, kind="ExternalInput").ap()
    w_d = nc.dram_tensor("w", (D, M), F32, kind="ExternalInput").ap()
    v_d = nc.dram_tensor("v", (P, DC), F32, kind="ExternalOutput").ap()
    sf_d = nc.dram_tensor("sf_scratch", (P, M), F32, kind="Internal").ap()

    with tile.TileContext(nc) as tc:
        with ExitStack() as ctx:
            _body(ctx, tc, v_d, u_d, w_d, sf_d)
    nc.compile()
    return nc


def _body(ctx, tc, v_d, u_d, w_d, sf_d):
    nc = tc.nc
    const = ctx.enter_context(tc.tile_pool(name="const", bufs=1))
    work = ctx.enter_context(tc.tile_pool(name="work", bufs=2))
    stage = ctx.enter_context(tc.tile_pool(name="stage", bufs=2))
    pquad = ctx.enter_context(tc.tile_pool(name="pquad", bufs=2, space="PSUM"))
    pmm = ctx.enter_context(tc.tile_pool(name="pmm", bufs=2, space="PSUM"))
    pbig = ctx.enter_context(tc.tile_pool(name="pbig", bufs=1, space="PSUM"))

    # ---------------- persistent SBUF state ----------------
    ident = const.tile([P, P], F32)
    make_identity(nc, ident)
    ident_r = const.tile([P, P], F32R)
    nc.gpsimd.tensor_copy(out=ident_r[:], in_=ident[:])
    eps_sb = const.tile([P, 1], F32)
    nc.gpsimd.memset(eps_sb[:].bitcast(F32), EPS)

    # block-masked all-ones weights for round 0 (uniform softmax):
    # onesm[bl] = [128, 128] with cols [32bl, 32bl+32) = 1, else 0
    onesm = const.tile([P, BL * P], F32R)
    nc.gpsimd.memset(onesm[:].bitcast(F32), 0.0)
    nc.gpsimd.memset(
        _ap(onesm[:], 0, [[BL * P, P], [P + 32, BL], [1, 32]]).bitcast(F32), 1.0)

    u_sb = const.tile([P, BL * 8 * D], F32R)   # u[bl][jk]: [128(j), 256(d)]
    uT_sb = const.tile([P, BL * 2 * N], F32R)  # uT[bl][dk]: [128(d), 1024(j)]
    w_sb = const.tile([P, 2 * M], F32R)        # w[dk]: [128(d), 2048(m)]
    wT_sb = const.tile([P, 16 * D], F32R)      # wT[mk]: [128(m), 256(d)]
    b_sb = const.tile([P, N], F32)             # routing logits
    # block-masked exp(b)^T: eTm[(bl,jk)][j_local, p] = e[p, jk*128+j_local]
    # for p in bl's block, else 0
    eTm = const.tile([P, BL * 8 * P], F32R)
    nc.gpsimd.memset(eTm[:].bitcast(F32), 0.0)
    # block-masked wv^T: wvm[(bl,dk)][d_local, p] masked to bl's block
    wvm = const.tile([P, BL * 2 * P], F32R)
    nc.gpsimd.memset(wvm[:].bitcast(F32), 0.0)
    vemb = const.tile([P, 16 * P], F32R)       # block-diag s embedding
    nc.gpsimd.memset(vemb[:].bitcast(F32), 0.0)

    copy_engines = [nc.scalar.copy, nc.vector.tensor_copy]

    def tr_quad(dst_ap, src_aps, eng, dt=F32):
        """Transpose up to 4 128x128 blocks into one psum tile, then one
        consolidated copy to a contiguous dst slice."""
        idn = ident if dt == F32 else ident_r
        pt = pquad.tile([P, len(src_aps) * P], dt, tag="quad")
        for q, sap in enumerate(src_aps):
            nc.tensor.transpose(out=pt[:, q * P:(q + 1) * P], in_=sap,
                                identity=idn[:])
        copy_engines[eng % 2](out=dst_ap, in_=pt[:])
        return pt

    # ---------------- load W + build W^T ----------------
    for dk in range(2):
        wst = stage.tile([P, M], F32, tag="wst")
        nc.sync.dma_start(out=wst[:], in_=w_d[dk * 128:(dk + 1) * 128, :])
        for half in range(2):
            copy_engines[half](
                out=w_sb[:, dk * M + half * 1024: dk * M + (half + 1) * 1024],
                in_=wst[:, half * 1024:(half + 1) * 1024])
        # wT dst cols for (mk, dk): mk*256 + dk*128 -> for fixed dk the
        # 16 mk-blocks are stride-256; write 4 transposes per strided copy
        for g in range(4):
            pt = pquad.tile([P, 4 * P], F32, tag="quad")
            for q in range(4):
                mk = g * 4 + q
                nc.tensor.transpose(out=pt[:, q * P:(q + 1) * P],
                                    in_=wst[:, mk * 128:(mk + 1) * 128],
                                    identity=ident[:])
            dst = _ap(wT_sb[:], (g * 4) * D + dk * 128,
                      [[16 * D, P], [D, 4], [1, P]])
            copy_engines[(dk * 4 + g) % 2](
                out=dst, in_=pt[:].rearrange("p (q c) -> p q c", q=4))

    # ---------------- load u (one DMA per local batch) + build u^T --------
    for bl in range(BL):
        ust = stage.tile([P, 8 * D], F32, tag="ust")
        # gather the 8 j-tiles of batch bl in one DMA:
        # dst[p, (jk, d)] = u[bl*1024 + jk*128 + p, d]
        src = _ap(u_d, bl * N * D, [[D, P], [P * D, 8], [1, D]])
        nc.sync.dma_start(out=ust[:].rearrange("p (jk d) -> p jk d", jk=8),
                          in_=src)
        for half in range(2):
            copy_engines[half](
                out=u_sb[:, bl * 8 * D + half * 1024:
                         bl * 8 * D + (half + 1) * 1024],
                in_=ust[:, half * 1024:(half + 1) * 1024])
        for dk in range(2):
            for g in range(2):
                srcs = [ust[:, (g * 4 + q) * D + dk * 128:
                            (g * 4 + q) * D + (dk + 1) * 128]
                        for q in range(4)]
                tr_quad(uT_sb[:, (bl * 2 + dk) * N + g * 512:
                              (bl * 2 + dk) * N + (g + 1) * 512],
                        srcs, eng=bl * 4 + dk * 2 + g)

    # ---------------- routing rounds ----------------
    for r in range(ROUTINGS):
        # cu[p, d] = sum_j e[p, j] * u[bl(p)][j, d] as one accumulation over
        # the concatenated (bl, jk) axis with block-masked weights
        cu_ps = pmm.tile([P, D], F32, tag="mm")
        first, last = (0, 0), (BL - 1, 7)
        for bl in range(BL):
            for jk in range(8):
                lhs = (onesm[:, bl * P:(bl + 1) * P] if r == 0 else
                       eTm[:, (bl * 8 + jk) * P:(bl * 8 + jk + 1) * P])
                nc.tensor.matmul(
                    out=cu_ps[:],
                    lhsT=lhs,
                    rhs=u_sb[:, (bl * 8 + jk) * D:(bl * 8 + jk + 1) * D],
                    start=((bl, jk) == first), stop=((bl, jk) == last))
        cu_sb = work.tile([P, D], F32R, tag="cu")
        nc.scalar.copy(out=cu_sb[:, 0:128], in_=cu_ps[:, 0:128])
        nc.vector.tensor_copy(out=cu_sb[:, 128:256], in_=cu_ps[:, 128:256])
        cuT_sb = work.tile([P, D], F32R, tag="cuT")
        tr_quad(cuT_sb[:], [cu_sb[:, 0:128], cu_sb[:, 128:256]], eng=r,
                dt=F32R)

        # s_full[p, m] = sum_d cu[p, d] * W[d, m]; two psum halves,
        # bounce each through DRAM to extract diagonal blocks:
        # s[p, d'] = s_full[p, i(p)*64 + d'],  i(p) = p % 32
        # (half h holds capsules [16h, 16h+16), so extract per half)
        s_sb = work.tile([P, DC], F32, tag="s")
        sf_ps = pbig.tile([P, M], F32, tag="big")
        for n in range(4):
            for dk in range(2):
                nc.tensor.matmul(
                    out=sf_ps[:, n * 512:(n + 1) * 512],
                    lhsT=cuT_sb[:, dk * 128:(dk + 1) * 128],
                    rhs=w_sb[:, dk * M + n * 512: dk * M + (n + 1) * 512],
                    start=(dk == 0), stop=(dk == 1))
        sf_sb = work.tile([P, M], F32, tag="sf")
        for q in range(2):
            copy_engines[q](out=sf_sb[:, q * 1024:(q + 1) * 1024],
                            in_=sf_ps[:, q * 1024:(q + 1) * 1024])
        nc.sync.dma_start(out=sf_d[:], in_=sf_sb[:])
        for bl in range(BL):
            src = _ap(sf_d, bl * 32 * M, [[M + DC, 32], [1, DC]])
            nc.sync.dma_start(out=s_sb[bl * 32:(bl + 1) * 32, :], in_=src)

        # squash scale: rv = 1/sqrt(sum(s^2) + eps)
        sq_sb = work.tile([P, DC], F32, tag="sq")
        ssq = work.tile([P, 1], F32, tag="ssq")
        nc.vector.scalar_tensor_tensor(out=sq_sb[:], in0=s_sb[:], scalar=1.0,
                                       in1=s_sb[:], op0=ALU.mult,
                                       op1=ALU.mult, accum_out=ssq[:])
        sr = work.tile([P, 1], F32, tag="sr")
        nc.scalar.activation(out=sr[:], in_=ssq[:], func=AF.Sqrt,
                             bias=eps_sb[:])
        rv = work.tile([P, 1], F32, tag="rv")
        nc.vector.reciprocal(out=rv[:], in_=sr[:])

        if r == ROUTINGS - 1:
            v_sb = work.tile([P, DC], F32, tag="v")
            nc.vector.tensor_scalar(out=v_sb[:], in0=s_sb[:],
                                    scalar1=rv[:, 0:1], scalar2=None,
                                    op0=ALU.mult)
            nc.sync.dma_start(out=v_d[:], in_=v_sb[:])
            continue

        # s2 = [s, s] duplicated along free dim; s2T[t*64+d', p] = s[p, d']
        s2_sb = work.tile([P, 2 * DC], F32, tag="s2")
        nc.scalar.copy(out=s2_sb[:].rearrange("p (t c) -> p t c", t=2),
                       in_=s_sb[:].unsqueeze(1).to_broadcast([P, 2, DC]))
        s2T_sb = work.tile([P, P], F32, tag="s2T")
        tr_quad(s2T_sb[:], [s2_sb[:]], eng=1)

        # scatter s into the block-diagonal embedding vemb[m_tile k]:
        # vemb_k[t*64+d', p] = s[p, d'] for p with capsule i(p) == 2k+t
        # one strided-AP copy per t
        for t in range(2):
            src = _ap(s2T_sb[:], t * 64 * P + t, [[P, 64], [2, 16], [32, 4]])
            dst = _ap(vemb[:], t * 64 * (16 * P) + t,
                      [[16 * P, 64], [P + 2, 16], [32, 4]])
            copy_engines[t](out=dst, in_=src)

        # w_v[p, d] = sum_{d'} s[p, d'] * W[d, i(p)*64+d']  (then scaled by rv)
        wv_ps = pmm.tile([P, D], F32, tag="mm")
        for k in range(16):
            nc.tensor.matmul(out=wv_ps[:],
                             lhsT=vemb[:, k * P:(k + 1) * P],
                             rhs=wT_sb[:, k * D:(k + 1) * D],
                             start=(k == 0), stop=(k == 15))
        wv_sb = work.tile([P, D], F32R, tag="wv")
        nc.vector.tensor_scalar(out=wv_sb[:], in0=wv_ps[:],
                                scalar1=rv[:, 0:1], scalar2=None, op0=ALU.mult)
        # transpose wv (both halves into one psum quad), then one 4-level-AP
        # copy scatters both dk blocks into the masked wvm tiles
        pt = pquad.tile([P, 2 * P], F32R, tag="quad")
        for dk in range(2):
            nc.tensor.transpose(out=pt[:, dk * P:(dk + 1) * P],
                                in_=wv_sb[:, dk * 128:(dk + 1) * 128],
                                identity=ident_r[:])
        dst = _ap(wvm[:], 0,
                  [[BL * 2 * P, P], [2 * P + 32, BL], [P, 2], [1, 32]])
        srcp = _ap(pt[:], 0, [[2 * P, P], [32, BL], [P, 2], [1, 32]])
        nc.vector.tensor_copy(out=dst, in_=srcp)

        # bu[p, j] = sum_d wv[p, d] * u[bl(p)][j, d] as one accumulation over
        # the concatenated (bl, dk) axis with block-masked weights
        bu_ps = pbig.tile([P, N], F32, tag="big")
        for n in range(2):
            for bl in range(BL):
                for dk in range(2):
                    nc.tensor.matmul(
                        out=bu_ps[:, n * 512:(n + 1) * 512],
                        lhsT=wvm[:, (bl * 2 + dk) * P:(bl * 2 + dk + 1) * P],
                        rhs=uT_sb[:, (bl * 2 + dk) * N + n * 512:
                                  (bl * 2 + dk) * N + (n + 1) * 512],
                        start=(bl == 0 and dk == 0), stop=(bl == 3 and dk == 1))

        # b += bu ; eTm[(bl,jk)] = masked exp(b)^T (exp fused into copy-out,
        # one strided-AP activation per jk); chunked so each transpose can
        # start as soon as its chunk of b is updated
        for g in range(2):
            gsl = slice(g * 512, (g + 1) * 512)
            if r == 0:
                nc.vector.tensor_copy(out=b_sb[:, gsl], in_=bu_ps[:, gsl])
            else:
                nc.vector.tensor_add(out=b_sb[:, gsl], in0=b_sb[:, gsl],
                                     in1=bu_ps[:, gsl])
            pt_f = pquad.tile([P, 4 * P], F32, tag="quad")
            for q in range(4):
                jk = g * 4 + q
                nc.tensor.transpose(out=pt_f[:, q * P:(q + 1) * P],
                                    in_=b_sb[:, jk * 128:(jk + 1) * 128],
                                    identity=ident[:])
            # eTm col for (bl, jk, c) = bl*1056 + jk*128 + c
            dst = _ap(eTm[:], g * 512,
                      [[BL * 8 * P, P], [8 * P + 32, BL], [P, 4], [1, 32]])
            srcp = _ap(pt_f[:], 0, [[4 * P, P], [32, BL], [P, 4], [1, 32]])
            nc.scalar.activation(out=dst, in_=srcp, func=AF.Exp)


_NC_CACHE = None


def _get_nc():
    global _NC_CACHE
    if _NC_CACHE is None:
        _NC_CACHE = _build_kernel()
    return _NC_CACHE


def kernel(u_vecs: np.ndarray, W: np.ndarray) -> np.ndarray:
    u_vecs = np.ascontiguousarray(np.asarray(u_vecs, dtype=np.float32))
    W0 = np.ascontiguousarray(np.asarray(W, dtype=np.float32).reshape(D, M))
    nc = _get_nc()
    in_maps = [
        {"u": u_vecs[c * BL:(c + 1) * BL].reshape(BL * N, D), "w": W0}
        for c in range(N_CORES)
    ]
    res = bass_utils.run_bass_kernel_spmd(nc, in_maps,
                                          core_ids=list(range(N_CORES)))
    out = np.empty((B, NC, DC), dtype=np.float32)
    for c in range(N_CORES):
        out[c * BL:(c + 1) * BL] = res.results[c]["v"].reshape(BL, NC, DC)
    return out
